# revision 2
# baseline (speedup 1.0000x reference)
"""Trainium2 Bass kernel v5 for EntropyAndMutualInformation.

v5 vs v4 (informed by PE microbenchmarks: stationary swap every matmul
costs ~161ns/mm vs ~116 with reuse; single-bank chains serialize):
  * TRANSPOSED output tiles: stationary = Y cols (rhs), moving = X slabs.
    Each stationary is reused across 2 consecutive matmuls (xc chunks) and
    consecutive matmuls hit different PSUM banks (8-way interleave).
  * margX computed during P1 via ones-matmuls over local X probs
    (mathematically identical to joint row-sums; PE is idle during P1).
  * margY from drain row-sums of transposed tiles (replaces ones-matmul).
  * mode "a": single AllGather, 4 xg-sets with full-k PSUM chains.
    mode "b": AG in `nch` chunks (k-outer), SBUF f32 accumulation across
    chunks (adds split DVE/Pool), drains from acc.
Host combine updated for the new marg normalizations:
  margx partial = sum_n SCALE*pX  -> mX = margx/(n*SCALE)
  margy partial = sum_c T[c,d]    -> mY = margy/(n*SCALE^2), s_t = sum margy
"""
import sys

sys.path.insert(0, "/opt/trn_rl_repo")

import numpy as np

N_TOTAL, C_DIM, N_CORES = 8192, 4096, 8
EPS = 1e-12
SCALE = 2048.0


def build_nc(n_total=N_TOTAL, C=C_DIM, ncores=N_CORES, mode="a", nch=4,
             debug=False, ex_bf16=False, big_slab=True):
    import concourse.bass as bass
    import concourse.tile as tile
    import concourse.mybir as mybir
    from concourse import bacc

    f32 = mybir.dt.float32
    bf16 = mybir.dt.bfloat16
    fp8 = mybir.dt.float8e4
    P = 128
    n_shard = n_total // ncores          # 1024
    W = C // ncores                      # 512
    row_tiles = n_shard // P             # 8
    k_tiles = n_total // P               # 64
    n_sets = 4                           # xg groups (1024 xc each)
    n_yb = W // P                        # 4
    assert row_tiles % nch == 0
    tpc = row_tiles // nch               # tiles per AG chunk (mode b)

    nc = bacc.Bacc("TRN2", target_bir_lowering=False, debug=debug,
                   enable_asserts=True, num_devices=ncores)

    xy_in = nc.dram_tensor("xy", [2 * n_shard, C], f32,
                           kind="ExternalInput").ap()
    x_in = xy_in[0:n_shard, :]
    y_in = xy_in[n_shard:2 * n_shard, :]

    out_all = nc.dram_tensor("out", [53, P, 1], f32,
                             kind="ExternalOutput").ap()
    zx_out = out_all[0:row_tiles]
    dx_out = out_all[row_tiles:2 * row_tiles]
    margx_out = out_all[16:48].rearrange("a p o -> o (a p)")  # [1, 4096]
    tlogt_out = out_all[48, :, :]
    margy_out = out_all[49:53]           # [4, 128, 1]

    agx_in = nc.dram_tensor("agx_in", [n_shard, C], fp8)
    if mode == "a":
        agx_out = [nc.dram_tensor("agx_out", [n_total, C], fp8,
                                  addr_space="Shared")]
        chunks = [(0, row_tiles)]
    else:
        agx_out = [nc.dram_tensor(f"agx_out{c}",
                                  [ncores * tpc * P, C], fp8,
                                  addr_space="Shared") for c in range(nch)]
        chunks = [(c * tpc, tpc) for c in range(nch)]

    a2a_in = nc.dram_tensor("a2a_in", [ncores, n_shard, W], fp8)
    a2a_out = nc.dram_tensor("a2a_out", [ncores, n_shard, W], fp8)

    Exp = mybir.ActivationFunctionType.Exp
    Ln = mybir.ActivationFunctionType.Ln
    Copy = mybir.ActivationFunctionType.Copy
    mult = mybir.AluOpType.mult
    add = mybir.AluOpType.add
    DR = mybir.MatmulPerfMode.DoubleRow

    rg = [list(range(ncores))]

    a2a_in_v = a2a_in[:].rearrange("j (t p) w -> t p j w", p=P)
    # lhs views per chunk: [p, kt, c] with kt local to chunk, ranks-major
    lhs_view = [agx_out[c][:].rearrange("(k p) c -> p k c", p=P)
                for c in range(len(chunks))]

    with tile.TileContext(nc) as tc:
        with (
            tc.tile_pool(name="pin", bufs=2) as pin,
            tc.tile_pool(name="pe_", bufs=2) as pe_,
            tc.tile_pool(name="ppr", bufs=2) as ppr,
            tc.tile_pool(name="pscr", bufs=1) as pscr,
            tc.tile_pool(name="p1s", bufs=8) as p1s,
            tc.tile_pool(name="rhsp", bufs=1) as rhsp,
            tc.tile_pool(name="constp", bufs=1) as constp,
            tc.tile_pool(name="slabp", bufs=4) as slabp,
            tc.tile_pool(name="jpsum", bufs=8, space="PSUM") as jpsum,
            tc.tile_pool(name="accp", bufs=1) as accp,
            tc.tile_pool(name="drainp", bufs=2) as drainp,
            tc.tile_pool(name="smallp", bufs=8) as smallp,
        ):
            ones3 = constp.tile([P, 1], fp8)
            nc.vector.memset(ones3[:], 1.0)
            ln_bias = constp.tile([P, 1], f32)
            nc.vector.memset(ln_bias[:], float(SCALE) * SCALE * n_total * EPS)

            rhs = rhsp.tile([P, k_tiles, W], fp8)

            def p1_y_tile(t):
                yt = pin.tile([P, C], f32, tag="xt")
                nc.scalar.dma_start(yt[:], y_in[t * P:(t + 1) * P, :])
                ey = pe_.tile([P, C], bf16, tag="et")
                zy = p1s.tile([P, 1], f32, tag="z")
                nc.scalar.activation(ey[:], yt[:], Exp, accum_out=zy[:])
                rzy = p1s.tile([P, 1], f32, tag="rz")
                nc.vector.reciprocal(rzy[:], zy[:])
                rzys = p1s.tile([P, 1], f32, tag="rzs")
                nc.vector.tensor_scalar_mul(rzys[:], rzy[:], float(SCALE))
                pyt = ppr.tile([P, C], fp8, tag="pt")
                nc.vector.tensor_scalar_mul(pyt[:], ey[:], rzys[:])
                nc.sync.dma_start(
                    a2a_in_v[t], pyt[:].rearrange("p (j w) -> p j w", j=ncores))

            mxp = [jpsum.tile([1, 512], f32, tag="jp", name=f"mxp{i}")
                   for i in range(8)]

            def p1_x_tile(t):
                xt = pin.tile([P, C], f32, tag="xt")
                nc.scalar.dma_start(xt[:], x_in[t * P:(t + 1) * P, :])
                ex = pe_.tile([P, C], bf16, tag="et")
                zx = p1s.tile([P, 1], f32, tag="z")
                nc.scalar.activation(ex[:], xt[:], Exp, accum_out=zx[:])
                nc.scalar.dma_start(zx_out[t], zx[:])
                rzx = p1s.tile([P, 1], f32, tag="rz")
                nc.vector.reciprocal(rzx[:], zx[:])
                rzxs = p1s.tile([P, 1], f32, tag="rzs")
                nc.vector.tensor_scalar_mul(rzxs[:], rzx[:], float(SCALE))
                pxt = ppr.tile([P, C], fp8, tag="pt")
                nc.scalar.activation(pxt[:], ex[:], Copy, scale=rzxs[:])
                nc.sync.dma_start(agx_in[t * P:(t + 1) * P, :], pxt[:])
                # margX partial: ones-matmul column sums of local probs
                for ch in range(8):
                    nc.tensor.matmul(mxp[ch][:], ones3[:, 0:1],
                                     pxt[:, ch * 512:(ch + 1) * 512],
                                     start=(t == 0), stop=(t == row_tiles - 1))
                scr = pscr.tile([P, C], bf16, tag="scr")
                dx = p1s.tile([P, 1], f32, tag="dx")
                nc.vector.scalar_tensor_tensor(
                    out=scr[:], in0=ex[:], scalar=1.0, in1=xt[:],
                    op0=mult, op1=mult, accum_out=dx[:])
                nc.sync.dma_start(dx_out[t], dx[:])

            # ---------------- pipeline ----------------
            for t in range(row_tiles):
                p1_y_tile(t)
            nc.gpsimd.collective_compute(
                "AllToAll", mybir.AluOpType.bypass, replica_groups=rg,
                ins=[a2a_in[:]], outs=[a2a_out[:]])
            for t in range(row_tiles):
                p1_x_tile(t)
            for c, (t0, tl) in enumerate(chunks):
                nc.gpsimd.collective_compute(
                    "AllGather", mybir.AluOpType.bypass, replica_groups=rg,
                    ins=[agx_in[t0 * P:(t0 + tl) * P, :]],
                    outs=[agx_out[c][:]])

            # margx psums -> out (free the psum banks before mm)
            margx_sb = constp.tile([1, C], f32)
            for ch in range(8):
                nc.vector.tensor_copy(margx_sb[:, ch * 512:(ch + 1) * 512],
                                      mxp[ch][:])
            nc.scalar.dma_start(margx_out[:], margx_sb[:])

            # rhs load (during AG window)
            for j in range(ncores):
                nc.sync.dma_start(
                    rhs[:, j * row_tiles:(j + 1) * row_tiles, :],
                    a2a_out[j, :, :].rearrange("(t p) w -> p t w", p=P))

            # ---- transposed joint matmul ----
            tl_prev = [None]
            my_acc = [None] * n_yb

            def drain_tile(src_t, yb):
                # src_t: [128 yc, 512 xc] tile (psum or sbuf f32)
                lnt = drainp.tile([P, 512], bf16, tag="lnt")
                nc.scalar.activation(lnt[:], src_t[:], Ln, bias=ln_bias[:])
                scr1 = drainp.tile([P, 512], bf16, tag="scr1")
                tt_tmp = smallp.tile([P, 1], f32, tag="tttmp")
                nc.vector.scalar_tensor_tensor(
                    out=scr1[:], in0=src_t[:], scalar=1.0,
                    in1=lnt[:], op0=mult, op1=mult, accum_out=tt_tmp[:])
                tl_new = smallp.tile([P, 1], f32, tag="acct")
                if tl_prev[0] is None:
                    nc.vector.tensor_copy(tl_new[:], tt_tmp[:])
                else:
                    nc.vector.tensor_add(tl_new[:], tl_prev[0][:], tt_tmp[:])
                tl_prev[0] = tl_new
                scr2 = drainp.tile([P, 512], bf16, tag="scr2")
                st_y = smallp.tile([P, 1], f32, tag="sty")
                nc.vector.tensor_scalar(
                    out=scr2[:], in0=src_t[:], scalar1=1.0,
                    scalar2=None, op0=mult, op1=add, accum_out=st_y[:])
                my_new = smallp.tile([P, 1], f32, tag="myacc")
                if my_acc[yb] is None:
                    nc.vector.tensor_copy(my_new[:], st_y[:])
                else:
                    nc.vector.tensor_add(my_new[:], my_acc[yb][:], st_y[:])
                my_acc[yb] = my_new

            if mode == "a":
                # sets outer, full-k chains (single AG chunk)
                lv = lhs_view[0]
                for s in range(n_sets):
                    psums = [jpsum.tile([P, 512], f32, tag="jp",
                                        name=f"jp_{s}_{b}") for b in range(8)]
                    for j in range(ncores):
                        kj = j * row_tiles
                        slab = slabp.tile([P, row_tiles, 1024], fp8,
                                          tag="slab")
                        nc.sync.dma_start(
                            slab[:], lv[:, kj:kj + row_tiles,
                                        s * 1024:(s + 1) * 1024])
                        for tp in range(row_tiles // 2):
                            kk = kj + 2 * tp
                            first = (j == 0 and tp == 0)
                            last = (j == ncores - 1 and
                                    tp == row_tiles // 2 - 1)
                            for yb in range(n_yb):
                                for xi in range(2):
                                    nc.tensor.matmul(
                                        psums[yb * 2 + xi][:],
                                        rhs[:, kk:kk + 2,
                                            yb * P:(yb + 1) * P],
                                        slab[:, 2 * tp:2 * tp + 2,
                                             xi * 512:(xi + 1) * 512],
                                        start=first, stop=last,
                                        perf_mode=DR)
                    for b in range(8):
                        drain_tile(psums[b], b // 2)
            else:
                accs = [accp.tile([P, 512], f32, name=f"acc_{s}_{b}")
                        for s in range(n_sets) for b in range(8)]
                for c in range(nch):
                    lv = lhs_view[c]
                    for s in range(n_sets):
                        psums = [jpsum.tile([P, 512], f32, tag="jp",
                                            name=f"jp_{c}_{s}_{b}")
                                 for b in range(8)]
                        for j in range(ncores):
                            for tp in range(tpc // 2):
                                kk = j * tpc + 2 * tp
                                slab = slabp.tile([P, 2, 1024], fp8,
                                                  tag="slab")
                                nc.sync.dma_start(
                                    slab[:], lv[:, kk:kk + 2,
                                                s * 1024:(s + 1) * 1024])
                                first = (j == 0 and tp == 0)
                                last = (j == ncores - 1 and tp == tpc // 2 - 1)
                                for yb in range(n_yb):
                                    for xi in range(2):
                                        gk = j * row_tiles + c * tpc + 2 * tp
                                        nc.tensor.matmul(
                                            psums[yb * 2 + xi][:],
                                            rhs[:, gk:gk + 2,
                                                yb * P:(yb + 1) * P],
                                            slab[:, :,
                                                 xi * 512:(xi + 1) * 512],
                                            start=first, stop=last,
                                            perf_mode=DR)
                        for b in range(8):
                            a = accs[s * 8 + b]
                            if c == 0:
                                nc.scalar.activation(a[:], psums[b][:], Copy)
                            else:
                                nc.vector.tensor_add(a[:], a[:], psums[b][:])
                            if c == nch - 1:
                                drain_tile(a, b // 2)

            nc.scalar.dma_start(tlogt_out[:], tl_prev[0][:])
            for yb in range(n_yb):
                nc.scalar.dma_start(margy_out[yb], my_acc[yb][:])

    nc.compile()
    return nc


_CACHE = {}


def _get_compiled(key=("a", 4)):
    if key not in _CACHE:
        _CACHE[key] = build_nc(mode=key[0], nch=key[1])
    return _CACHE[key]


def combine_host(results, n_total=N_TOTAL, C=C_DIM, ncores=N_CORES):
    n = float(n_total)
    s = SCALE
    s2 = s * s
    ent_sum = 0.0
    s_tln = 0.0
    margx = np.zeros(C, dtype=np.float64)
    margy_blocks = []
    for r in results:
        o = r["out"].astype(np.float64).reshape(53, 128)
        z = o[0:8].ravel()
        d = o[8:16].ravel()
        ent_sum += np.sum(np.log(z) - d / z)
        s_tln += float(np.sum(o[48]))
        margx += o[16:48].reshape(-1)
        margy_blocks.append(o[49:53].ravel())
    margy = np.concatenate(margy_blocks)
    s_t = float(margy.sum())
    entropy = ent_sum / n
    S1 = ((s_tln - np.log(s2) * s_t) / s2 - np.log(n) * (s_t / s2)) / n
    mX = margx / (n * s)
    mY = margy / (n * s2)
    mi = S1 - np.sum(mX * np.log(mX + EPS)) - np.sum(mY * np.log(mY + EPS))
    return np.array([entropy, mi], dtype=np.float32)


def kernel(act_X, act_Y):
    from concourse.bass_utils import run_bass_kernel_spmd

    act_X = np.ascontiguousarray(np.asarray(act_X, dtype=np.float32))
    act_Y = np.ascontiguousarray(np.asarray(act_Y, dtype=np.float32))
    assert act_X.shape == (N_TOTAL, C_DIM) and act_Y.shape == (N_TOTAL, C_DIM)

    nc = _get_compiled()
    n_shard = N_TOTAL // N_CORES
    in_maps = [
        {"xy": np.concatenate([act_X[k * n_shard:(k + 1) * n_shard],
                               act_Y[k * n_shard:(k + 1) * n_shard]], axis=0)}
        for k in range(N_CORES)
    ]
    res = run_bass_kernel_spmd(nc, in_maps, list(range(N_CORES)))
    return combine_host(res.results)
